# revision 4
# baseline (speedup 1.0000x reference)
"""MoE layer (D=1024, F=4096, E=8, top-2, T=4096 tokens) on 8 TRN2 NeuronCores.

Strategy (expert-parallel, per the sharding hint):
  Phase 1 (device, data-parallel): each core computes router logits for T/8=512
    tokens (fp32 PE matmul), then top-2 indices + renormalized weights on DVE
    (vector.max / max_index) and ACT (exp).
  Host dispatch (the "all-to-all"): using the *device-computed* indices, tokens
    are sharded by expert id; core e receives the gathered+transposed activations
    xgT[:, :n_e] for its expert, padded to a fixed capacity CAP.
  Phase 2 (device, expert-parallel): core e computes
    yT = w2_e^T @ relu(w1_e^T x + b1_e) scaled per-token by the combine weight.
    Layouts are chosen so both GEMMs use native weight layouts as the stationary
    operand and need zero on-device transposes:
      GEMM1: hT[F,t] : lhsT = w1[D,F] tile, rhs = xgT[D,t] tile
      GEMM2: yT[D,t] : lhsT = w2[F,D] tile, rhs = hT[F,t] tile
  Host combines: out[tok] += yT.T rows (unique tokens per core), plus the
    (identically zero) b2 term reconstructed exactly as the reference does.
"""

import os
import numpy as np

import concourse.bass as bass
import concourse.bacc as bacc
import concourse.mybir as mybir
from concourse.bass import ts
from concourse.tile import TileContext
from concourse.bass_utils import run_bass_kernel_spmd

F32 = mybir.dt.float32
I32 = mybir.dt.int32
U32 = mybir.dt.uint32

D = 1024
FF = 4096
E = 8
TOPK = 2
B, S = 2, 2048
T = B * S
NCORES = 8
TPC = T // NCORES  # tokens per core in phase 1 (router)

# Fixed capacity per expert (max observed count for the fixed key-0 inputs is
# 1091; padded to a multiple of 128). kernel() rebuilds with a larger CAP if a
# count ever exceeds this.
CAP_DEFAULT = 1152

KT_D = D // 128    # 8  k-tiles over D
FT = FF // 128     # 32 f-tiles over F
DT = D // 128      # 8  d-tiles over D


def _token_ntiles(cap):
    """Split cap tokens into moving-operand tiles of <=512."""
    out = []
    off = 0
    while off < cap:
        w = min(512, cap - off)
        out.append((off, w))
        off += w
    return out


def build_phase1():
    nc = bacc.Bacc("TRN2", target_bir_lowering=False)
    xT = nc.dram_tensor("xT", [D, TPC], F32, kind="ExternalInput")
    gate = nc.dram_tensor("gate", [D, E], F32, kind="ExternalInput")
    logits_out = nc.dram_tensor("logits_out", [TPC // 128, 128, E], F32,
                                kind="ExternalOutput")
    idx_out = nc.dram_tensor("idx_out", [TPC // 128, 128, TOPK], I32,
                             kind="ExternalOutput")
    wts_out = nc.dram_tensor("wts_out", [TPC // 128, 128, TOPK], F32,
                             kind="ExternalOutput")
    MT = TPC // 128  # 4 token m-tiles

    with TileContext(nc) as tc:
        with (
            tc.tile_pool(name="sb", bufs=1) as sb,
            tc.tile_pool(name="work", bufs=2) as work,
            tc.tile_pool(name="ps", bufs=4, space="PSUM") as psp,
        ):
            xT_sb = sb.tile([128, KT_D, TPC], F32)
            nc.sync.dma_start(xT_sb, xT.rearrange("(k p) t -> p k t", p=128))
            gate_sb = sb.tile([128, KT_D, E], F32)
            nc.sync.dma_start(gate_sb, gate.rearrange("(k p) e -> p k e", p=128))

            logits_sb = sb.tile([128, MT, E], F32)
            for m in range(MT):
                ps = psp.tile([128, E], F32, tag="logits")
                for k in range(KT_D):
                    nc.tensor.matmul(ps,
                                     lhsT=xT_sb[:, k, ts(m, 128)],
                                     rhs=gate_sb[:, k, :],
                                     start=(k == 0), stop=(k == KT_D - 1))
                nc.vector.tensor_copy(logits_sb[:, m, :], ps)
                nc.sync.dma_start(logits_out[m], logits_sb[:, m, :])

                mx = work.tile([128, 8], F32, tag="mx")
                mi = work.tile([128, 8], U32, tag="mi")
                nc.vector.max(mx, logits_sb[:, m, :])
                nc.vector.max_index(mi, mx, logits_sb[:, m, :])

                idx_pair = work.tile([128, TOPK], I32, tag="idx")
                nc.vector.tensor_copy(idx_pair, mi[:, 0:TOPK])
                nc.sync.dma_start(idx_out[m], idx_pair)

                # top-2 weights renormalized: w1 = 1/(1+exp(l2-l1)), w2 = 1-w1
                dneg = work.tile([128, 1], F32, tag="dneg")
                nc.vector.tensor_sub(dneg, mx[:, 1:2], mx[:, 0:1])
                ex = work.tile([128, 1], F32, tag="ex")
                nc.scalar.activation(ex, dneg, mybir.ActivationFunctionType.Exp)
                den = work.tile([128, 1], F32, tag="den")
                nc.vector.tensor_scalar_add(den, ex, 1.0)
                wts_pair = work.tile([128, TOPK], F32, tag="wts")
                nc.vector.reciprocal(wts_pair[:, 0:1], den)
                nc.vector.tensor_mul(wts_pair[:, 1:2], ex, wts_pair[:, 0:1])
                nc.sync.dma_start(wts_out[m], wts_pair)
    nc.compile()
    return nc


def build_phase2(cap):
    nc = bacc.Bacc("TRN2", target_bir_lowering=False)
    xgT = nc.dram_tensor("xgT", [D, cap], F32, kind="ExternalInput")
    w1 = nc.dram_tensor("w1", [D, FF], F32, kind="ExternalInput")
    w2 = nc.dram_tensor("w2", [FF, D], F32, kind="ExternalInput")
    b1 = nc.dram_tensor("b1", [FF], F32, kind="ExternalInput")
    cwb = nc.dram_tensor("cwb", [128, cap], F32, kind="ExternalInput")
    yT_out = nc.dram_tensor("yT_out", [D, cap], F32, kind="ExternalOutput")

    ntiles = _token_ntiles(cap)

    with TileContext(nc) as tc:
        with (
            tc.tile_pool(name="persist", bufs=1) as persist,
            tc.tile_pool(name="ps1", bufs=2, space="PSUM") as ps1p,
        ):
            hT = persist.tile([128, FT, cap], F32)
            cwb_sb = persist.tile([128, cap], F32)
            nc.sync.dma_start(cwb_sb, cwb[:, :])
            b1_sb = persist.tile([128, FT], F32)
            nc.sync.dma_start(b1_sb, b1.rearrange("(f p) -> p f", p=128))

            # ---- GEMM1: hT[f, t] = relu(w1.T @ xg + b1) ----
            with (
                tc.tile_pool(name="xg", bufs=1) as xgp,
                tc.tile_pool(name="w1p", bufs=3) as w1p,
            ):
                xg = xgp.tile([128, KT_D, cap], F32)
                nc.sync.dma_start(xg, xgT.rearrange("(k p) t -> p k t", p=128))
                for f in range(FT):
                    w1f = w1p.tile([128, KT_D, 128], F32, tag="w1f")
                    nc.sync.dma_start(
                        w1f, w1[:, ts(f, 128)].rearrange("(k p) c -> p k c", p=128))
                    for (off, w) in ntiles:
                        ps = ps1p.tile([128, 512], F32, tag="ps1")
                        for k in range(KT_D):
                            nc.tensor.matmul(ps[:, :w],
                                             lhsT=w1f[:, k, :],
                                             rhs=xg[:, k, off:off + w],
                                             start=(k == 0), stop=(k == KT_D - 1))
                        nc.scalar.activation(hT[:, f, off:off + w], ps[:, :w],
                                             mybir.ActivationFunctionType.Relu,
                                             bias=b1_sb[:, f:f + 1])

            # ---- GEMM2: yT[d, t] = (w2.T @ hT) * combine ----
            with (
                tc.tile_pool(name="w2p", bufs=2) as w2p,
                tc.tile_pool(name="yev", bufs=3) as yev,
                tc.tile_pool(name="ps2", bufs=2, space="PSUM") as ps2p,
            ):
                for d in range(DT):
                    w2d = w2p.tile([128, FT, 128], F32, tag="w2d")
                    nc.sync.dma_start(
                        w2d, w2[:, ts(d, 128)].rearrange("(k p) c -> p k c", p=128))
                    for (off, w) in ntiles:
                        ps = ps2p.tile([128, 512], F32, tag="ps2")
                        for kf in range(FT):
                            nc.tensor.matmul(ps[:, :w],
                                             lhsT=w2d[:, kf, :],
                                             rhs=hT[:, kf, off:off + w],
                                             start=(kf == 0), stop=(kf == FT - 1))
                        yt = yev.tile([128, 512], F32, tag="yt")
                        nc.vector.tensor_mul(yt[:, :w], ps[:, :w],
                                             cwb_sb[:, off:off + w])
                        nc.sync.dma_start(yT_out[ts(d, 128), off:off + w],
                                          yt[:, :w])
    nc.compile()
    return nc


_NC_CACHE = {}
LAST_EXEC_NS = {}
LAST_RESULTS = {}


def _get_nc(key, builder, *args):
    if key not in _NC_CACHE:
        _NC_CACHE[key] = builder(*args)
    return _NC_CACHE[key]


def _install_ntff_shim():
    """The agent image's antenv lacks axon_hooks, so boot() skipped NTFF hook
    registration. Recreate the module and register the ctypes-based hook."""
    import sys
    import types
    try:
        from antenv.axon_hooks import get_axon_ntff_profile_hook  # noqa: F401
        return True  # already present
    except ImportError:
        pass
    try:
        import antenv
        mod = types.ModuleType("antenv.axon_hooks")
        _holder = [None]
        mod.set_axon_ntff_profile_hook = lambda h: _holder.__setitem__(0, h)
        mod.get_axon_ntff_profile_hook = lambda: _holder[0]
        sys.modules["antenv.axon_hooks"] = mod
        antenv.axon_hooks = mod
        from trn_agent_boot.trn_boot import _ntff_profile_via_ctypes
        hook = _ntff_profile_via_ctypes("/opt/axon/libaxon_pjrt.so")
        if hook is None:
            return False
        mod.set_axon_ntff_profile_hook(hook)
        # artifact upload needs bucket creds we don't have; neuter it
        import concourse.bass_utils as bu
        bu.upload_artifacts = lambda tmpdir: tmpdir
        return True
    except Exception as e:  # pragma: no cover
        print(f"ntff shim install failed: {e}")
        return False


def _run(nc, in_maps, label):
    trace = bool(int(os.environ.get("MOE_TRACE", "0")))
    if trace and _install_ntff_shim():
        try:
            tmpdir = f"/tmp/moe_trace_{label}"
            os.makedirs(tmpdir, exist_ok=True)
            res = run_bass_kernel_spmd(
                nc, in_maps, core_ids=list(range(NCORES)),
                trace=True, trace_cores=list(range(NCORES)), tmpdir=tmpdir)
            LAST_EXEC_NS[label] = res.exec_time_ns
            LAST_EXEC_NS[label + "_mean"] = res.mean_exec_time_ns
            LAST_RESULTS[label] = res
            return res.results
        except Exception as e:
            print(f"traced run failed ({e!r}); retrying without trace")
    res = run_bass_kernel_spmd(nc, in_maps, core_ids=list(range(NCORES)))
    LAST_EXEC_NS[label] = res.exec_time_ns
    LAST_RESULTS[label] = res
    return res.results


def kernel(x, gate_w, w1, b1, w2, b2):
    x = np.ascontiguousarray(np.asarray(x, dtype=np.float32))
    gate_w = np.ascontiguousarray(np.asarray(gate_w, dtype=np.float32))
    w1 = np.ascontiguousarray(np.asarray(w1, dtype=np.float32))
    b1 = np.ascontiguousarray(np.asarray(b1, dtype=np.float32))
    w2 = np.ascontiguousarray(np.asarray(w2, dtype=np.float32))
    b2 = np.ascontiguousarray(np.asarray(b2, dtype=np.float32))

    x_flat = x.reshape(T, D)
    xT = np.ascontiguousarray(x_flat.T)  # [D, T]

    # ---- phase 1: router on device (data-parallel over tokens) ----
    nc1 = _get_nc("p1", build_phase1)
    in_maps = []
    for c in range(NCORES):
        in_maps.append({
            "xT": np.ascontiguousarray(xT[:, c * TPC:(c + 1) * TPC]),
            "gate": gate_w,
        })
    res1 = _run(nc1, in_maps, "phase1")

    logits = np.concatenate(
        [r["logits_out"].reshape(TPC, E) for r in res1], axis=0)
    idx = np.concatenate([r["idx_out"].reshape(TPC, TOPK) for r in res1], axis=0)
    wts = np.concatenate([r["wts_out"].reshape(TPC, TOPK) for r in res1], axis=0)

    # ---- host dispatch (sharding by expert id, using device-computed routing)
    toks, cws, counts = [], [], []
    for e in range(E):
        sel = (idx[:, 0] == e) | (idx[:, 1] == e)
        tok = np.nonzero(sel)[0]
        c_e = np.where(idx[tok, 0] == e, wts[tok, 0], wts[tok, 1])
        toks.append(tok)
        cws.append(c_e.astype(np.float32))
        counts.append(len(tok))
    cap = CAP_DEFAULT
    if max(counts) > cap:
        cap = ((max(counts) + 127) // 128) * 128

    nc2 = _get_nc(("p2", cap), build_phase2, cap)
    in_maps = []
    for e in range(E):
        n_e = counts[e]
        xgT = np.zeros((D, cap), dtype=np.float32)
        xgT[:, :n_e] = x_flat[toks[e]].T
        cw = np.zeros(cap, dtype=np.float32)
        cw[:n_e] = cws[e]
        in_maps.append({
            "xgT": xgT,
            "w1": w1[e],
            "w2": w2[e],
            "b1": b1[e],
            "cwb": np.ascontiguousarray(
                np.broadcast_to(cw, (128, cap))),
        })
    res2 = _run(nc2, in_maps, "phase2")

    # ---- host combine (scatter-add; token sets are disjoint per core) ----
    out_flat = np.zeros((T, D), dtype=np.float32)
    for e in range(E):
        n_e = counts[e]
        y = res2[e]["yT_out"].T  # [cap, D]
        out_flat[toks[e]] += y[:n_e]
        if np.any(b2[e]):
            out_flat[toks[e]] += cws[e][:, None] * b2[e][None, :]

    output = out_flat.reshape(B, S, D)
    router_logits = logits.reshape(B, S, E)
    top_k_indices = idx.reshape(B, S, TOPK).astype(np.int32)
    top_k_weights = wts.reshape(B, S, TOPK)
    return output, router_logits, top_k_indices, top_k_weights


# revision 9
# speedup vs baseline: 2.7660x; 2.7660x over previous
"""MoE layer (D=1024, F=4096, E=8, top-2, T=4096 tokens) on 8 TRN2 NeuronCores.

Strategy (expert-parallel, per the sharding hint):
  Phase 1 (device, data-parallel): each core computes router logits for T/8=512
    tokens (fp32 PE matmul), then top-2 indices + renormalized weights on DVE
    (vector.max / max_index) and ACT (exp).
  Host dispatch (the "all-to-all"): using the *device-computed* indices, tokens
    are sharded by expert id; core e receives the gathered+transposed activations
    xgT[:, :n_e] for its expert, padded to a fixed capacity CAP.
  Phase 2 (device, expert-parallel): core e computes
    yT = w2_e^T @ relu(w1_e^T x + b1_e) scaled per-token by the combine weight.
    Layouts are chosen so both GEMMs use native weight layouts as the stationary
    operand and need zero on-device transposes:
      GEMM1: hT[F,t] : lhsT = w1[D,F] tile, rhs = xgT[D,t] tile
      GEMM2: yT[D,t] : lhsT = w2[F,D] tile, rhs = hT[F,t] tile
  Host combines: out[tok] += yT.T rows (unique tokens per core), plus the
    (identically zero) b2 term reconstructed exactly as the reference does.
"""

import os
import numpy as np

import concourse.bass as bass
import concourse.bacc as bacc
import concourse.mybir as mybir
from concourse.bass import ts
from concourse.tile import TileContext
from concourse.bass_utils import run_bass_kernel_spmd

F32 = mybir.dt.float32
F32R = mybir.dt.float32r
I32 = mybir.dt.int32
U32 = mybir.dt.uint32

D = 1024
FF = 4096
E = 8
TOPK = 2
B, S = 2, 2048
T = B * S
NCORES = 8
TPC = T // NCORES  # tokens per core in phase 1 (router)

# Fixed capacity per expert (max observed count for the fixed key-0 inputs is
# 1091; padded to a multiple of 128). kernel() rebuilds with a larger CAP if a
# count ever exceeds this.
CAP_DEFAULT = 1152

KT_D = D // 128    # 8  k-tiles over D
FT = FF // 128     # 32 f-tiles over F
DT = D // 128      # 8  d-tiles over D


def _token_ntiles(cap):
    """Split cap tokens into moving-operand tiles of <=512."""
    out = []
    off = 0
    while off < cap:
        w = min(512, cap - off)
        out.append((off, w))
        off += w
    return out


def build_phase1():
    nc = bacc.Bacc("TRN2", target_bir_lowering=False)
    xT = nc.dram_tensor("xT", [D, TPC], F32, kind="ExternalInput")
    gate = nc.dram_tensor("gate", [D, E], F32, kind="ExternalInput")
    logits_out = nc.dram_tensor("logits_out", [TPC // 128, 128, E], F32,
                                kind="ExternalOutput")
    idx_out = nc.dram_tensor("idx_out", [TPC // 128, 128, TOPK], I32,
                             kind="ExternalOutput")
    wts_out = nc.dram_tensor("wts_out", [TPC // 128, 128, TOPK], F32,
                             kind="ExternalOutput")
    MT = TPC // 128  # 4 token m-tiles

    with TileContext(nc) as tc:
        with (
            tc.tile_pool(name="sb", bufs=1) as sb,
            tc.tile_pool(name="work", bufs=2) as work,
            tc.tile_pool(name="ps", bufs=4, space="PSUM") as psp,
        ):
            xT_sb = sb.tile([128, KT_D, TPC], F32)
            nc.sync.dma_start(xT_sb, xT.rearrange("(k p) t -> p k t", p=128))
            gate_sb = sb.tile([128, KT_D, E], F32)
            nc.sync.dma_start(gate_sb, gate.rearrange("(k p) e -> p k e", p=128))

            logits_sb = sb.tile([128, MT, E], F32)
            for m in range(MT):
                ps = psp.tile([128, E], F32, tag="logits")
                for k in range(KT_D):
                    nc.tensor.matmul(ps,
                                     lhsT=xT_sb[:, k, ts(m, 128)],
                                     rhs=gate_sb[:, k, :],
                                     start=(k == 0), stop=(k == KT_D - 1))
                nc.vector.tensor_copy(logits_sb[:, m, :], ps)
                nc.sync.dma_start(logits_out[m], logits_sb[:, m, :])

                mx = work.tile([128, 8], F32, tag="mx")
                mi = work.tile([128, 8], U32, tag="mi")
                nc.vector.max(mx, logits_sb[:, m, :])
                nc.vector.max_index(mi, mx, logits_sb[:, m, :])

                idx_pair = work.tile([128, TOPK], I32, tag="idx")
                nc.vector.tensor_copy(idx_pair, mi[:, 0:TOPK])
                nc.sync.dma_start(idx_out[m], idx_pair)

                # top-2 weights renormalized: w1 = 1/(1+exp(l2-l1)), w2 = 1-w1
                dneg = work.tile([128, 1], F32, tag="dneg")
                nc.vector.tensor_sub(dneg, mx[:, 1:2], mx[:, 0:1])
                ex = work.tile([128, 1], F32, tag="ex")
                nc.scalar.activation(ex, dneg, mybir.ActivationFunctionType.Exp)
                den = work.tile([128, 1], F32, tag="den")
                nc.vector.tensor_scalar_add(den, ex, 1.0)
                wts_pair = work.tile([128, TOPK], F32, tag="wts")
                nc.vector.reciprocal(wts_pair[:, 0:1], den)
                nc.vector.tensor_mul(wts_pair[:, 1:2], ex, wts_pair[:, 0:1])
                nc.sync.dma_start(wts_out[m], wts_pair)
    nc.compile()
    return nc


def build_phase2(cap):
    nc = bacc.Bacc("TRN2", target_bir_lowering=False)
    xgT = nc.dram_tensor("xgT", [D, cap], F32, kind="ExternalInput")
    w1 = nc.dram_tensor("w1", [D, FF], F32, kind="ExternalInput")
    w2 = nc.dram_tensor("w2", [FF, D], F32, kind="ExternalInput")
    b1 = nc.dram_tensor("b1", [FF], F32, kind="ExternalInput")
    cwb = nc.dram_tensor("cwb", [128, cap], F32, kind="ExternalInput")
    yT_out = nc.dram_tensor("yT_out", [D, cap], F32, kind="ExternalOutput")

    ntiles = _token_ntiles(cap)

    with TileContext(nc) as tc:
        with (
            tc.tile_pool(name="persist", bufs=1) as persist,
            tc.tile_pool(name="ps1", bufs=2, space="PSUM") as ps1p,
        ):
            hT = persist.tile([128, FT, cap], F32R)
            cwb_sb = persist.tile([128, cap], F32)
            nc.sync.dma_start(cwb_sb, cwb[:, :])
            b1_sb = persist.tile([128, FT], F32)
            nc.sync.dma_start(b1_sb, b1.rearrange("(f p) -> p f", p=128))

            # ---- GEMM1: hT[f, t] = relu(w1.T @ xg + b1) ----
            with (
                tc.tile_pool(name="xg", bufs=1) as xgp,
                tc.tile_pool(name="w1p", bufs=3) as w1p,
            ):
                xg = xgp.tile([128, KT_D, cap], F32R)
                nc.sync.dma_start(xg, xgT.rearrange("(k p) t -> p k t", p=128).bitcast(F32R))
                for f in range(FT):
                    w1f = w1p.tile([128, KT_D, 128], F32R, tag="w1f")
                    nc.sync.dma_start(
                        w1f, w1[:, ts(f, 128)].rearrange("(k p) c -> p k c", p=128).bitcast(F32R))
                    for (off, w) in ntiles:
                        ps = ps1p.tile([128, 512], F32, tag="ps1")
                        for k in range(KT_D):
                            nc.tensor.matmul(ps[:, :w],
                                             lhsT=w1f[:, k, :],
                                             rhs=xg[:, k, off:off + w],
                                             start=(k == 0), stop=(k == KT_D - 1))
                        nc.scalar.activation(hT[:, f, off:off + w], ps[:, :w],
                                             mybir.ActivationFunctionType.Relu,
                                             bias=b1_sb[:, f:f + 1])

            # ---- GEMM2: yT[d, t] = (w2.T @ hT) * combine ----
            with (
                tc.tile_pool(name="w2p", bufs=2) as w2p,
                tc.tile_pool(name="yev", bufs=3) as yev,
                tc.tile_pool(name="ps2", bufs=2, space="PSUM") as ps2p,
            ):
                for d in range(DT):
                    w2d = w2p.tile([128, FT, 128], F32R, tag="w2d")
                    nc.sync.dma_start(
                        w2d, w2[:, ts(d, 128)].rearrange("(k p) c -> p k c", p=128).bitcast(F32R))
                    for (off, w) in ntiles:
                        ps = ps2p.tile([128, 512], F32, tag="ps2")
                        for kf in range(FT):
                            nc.tensor.matmul(ps[:, :w],
                                             lhsT=w2d[:, kf, :],
                                             rhs=hT[:, kf, off:off + w],
                                             start=(kf == 0), stop=(kf == FT - 1))
                        yt = yev.tile([128, 512], F32, tag="yt")
                        nc.vector.tensor_mul(yt[:, :w], ps[:, :w],
                                             cwb_sb[:, off:off + w])
                        nc.sync.dma_start(yT_out[ts(d, 128), off:off + w],
                                          yt[:, :w])
    nc.compile()
    return nc


_NC_CACHE = {}
LAST_EXEC_NS = {}
LAST_RESULTS = {}


def _get_nc(key, builder, *args):
    if key not in _NC_CACHE:
        _NC_CACHE[key] = builder(*args)
    return _NC_CACHE[key]


def _install_ntff_shim():
    """The agent image's antenv lacks axon_hooks, so boot() skipped NTFF hook
    registration. Recreate the module and register the ctypes-based hook."""
    import sys
    import types
    try:
        from antenv.axon_hooks import get_axon_ntff_profile_hook  # noqa: F401
        return True  # already present
    except ImportError:
        pass
    try:
        import antenv
        mod = types.ModuleType("antenv.axon_hooks")
        _holder = [None]
        mod.set_axon_ntff_profile_hook = lambda h: _holder.__setitem__(0, h)
        mod.get_axon_ntff_profile_hook = lambda: _holder[0]
        sys.modules["antenv.axon_hooks"] = mod
        antenv.axon_hooks = mod
        from trn_agent_boot.trn_boot import _ntff_profile_via_ctypes
        hook = _ntff_profile_via_ctypes("/opt/axon/libaxon_pjrt.so")
        if hook is None:
            return False
        mod.set_axon_ntff_profile_hook(hook)
        # artifact upload needs bucket creds we don't have; neuter it
        import concourse.bass_utils as bu
        bu.upload_artifacts = lambda tmpdir: tmpdir
        return True
    except Exception as e:  # pragma: no cover
        print(f"ntff shim install failed: {e}")
        return False


def _run(nc, in_maps, label):
    trace = bool(int(os.environ.get("MOE_TRACE", "0")))
    if trace and _install_ntff_shim():
        try:
            import tempfile
            tmpdir = tempfile.mkdtemp(prefix=f"moe_trace_{label}_")
            res = run_bass_kernel_spmd(
                nc, in_maps, core_ids=list(range(NCORES)),
                trace=True, trace_cores=list(range(NCORES)), tmpdir=tmpdir)
            LAST_EXEC_NS[label] = res.exec_time_ns
            LAST_EXEC_NS[label + "_mean"] = res.mean_exec_time_ns
            LAST_RESULTS[label] = res
            return res.results
        except Exception as e:
            print(f"traced run failed ({e!r}); retrying without trace")
    res = run_bass_kernel_spmd(nc, in_maps, core_ids=list(range(NCORES)))
    LAST_EXEC_NS[label] = res.exec_time_ns
    LAST_RESULTS[label] = res
    return res.results


def kernel(x, gate_w, w1, b1, w2, b2):
    x = np.ascontiguousarray(np.asarray(x, dtype=np.float32))
    gate_w = np.ascontiguousarray(np.asarray(gate_w, dtype=np.float32))
    w1 = np.ascontiguousarray(np.asarray(w1, dtype=np.float32))
    b1 = np.ascontiguousarray(np.asarray(b1, dtype=np.float32))
    w2 = np.ascontiguousarray(np.asarray(w2, dtype=np.float32))
    b2 = np.ascontiguousarray(np.asarray(b2, dtype=np.float32))

    x_flat = x.reshape(T, D)
    xT = np.ascontiguousarray(x_flat.T)  # [D, T]

    # ---- phase 1: router on device (data-parallel over tokens) ----
    nc1 = _get_nc("p1", build_phase1)
    in_maps = []
    for c in range(NCORES):
        in_maps.append({
            "xT": np.ascontiguousarray(xT[:, c * TPC:(c + 1) * TPC]),
            "gate": gate_w,
        })
    res1 = _run(nc1, in_maps, "phase1")

    logits = np.concatenate(
        [r["logits_out"].reshape(TPC, E) for r in res1], axis=0)
    idx = np.concatenate([r["idx_out"].reshape(TPC, TOPK) for r in res1], axis=0)
    wts = np.concatenate([r["wts_out"].reshape(TPC, TOPK) for r in res1], axis=0)

    # ---- host dispatch (sharding by expert id, using device-computed routing)
    toks, cws, counts = [], [], []
    for e in range(E):
        sel = (idx[:, 0] == e) | (idx[:, 1] == e)
        tok = np.nonzero(sel)[0]
        c_e = np.where(idx[tok, 0] == e, wts[tok, 0], wts[tok, 1])
        toks.append(tok)
        cws.append(c_e.astype(np.float32))
        counts.append(len(tok))
    cap = CAP_DEFAULT
    if max(counts) > cap:
        cap = ((max(counts) + 127) // 128) * 128

    nc2 = _get_nc(("p2", cap), build_phase2, cap)
    in_maps = []
    for e in range(E):
        n_e = counts[e]
        xgT = np.zeros((D, cap), dtype=np.float32)
        xgT[:, :n_e] = x_flat[toks[e]].T
        cw = np.zeros(cap, dtype=np.float32)
        cw[:n_e] = cws[e]
        in_maps.append({
            "xgT": xgT,
            "w1": w1[e],
            "w2": w2[e],
            "b1": b1[e],
            "cwb": np.ascontiguousarray(
                np.broadcast_to(cw, (128, cap))),
        })
    res2 = _run(nc2, in_maps, "phase2")

    # ---- host combine (scatter-add; token sets are disjoint per core) ----
    out_flat = np.zeros((T, D), dtype=np.float32)
    for e in range(E):
        n_e = counts[e]
        y = res2[e]["yT_out"].T  # [cap, D]
        out_flat[toks[e]] += y[:n_e]
        if np.any(b2[e]):
            out_flat[toks[e]] += cws[e][:, None] * b2[e][None, :]

    output = out_flat.reshape(B, S, D)
    router_logits = logits.reshape(B, S, E)
    top_k_indices = idx.reshape(B, S, TOPK).astype(np.int32)
    top_k_weights = wts.reshape(B, S, TOPK)
    return output, router_logits, top_k_indices, top_k_weights
